# revision 1
# baseline (speedup 1.0000x reference)
"""BioRNN Trainium2 kernel (dev module).

Sharding: time x8 (125-step output windows, full batch 64 per core).
The leak (0.8/step) makes the state forget: starting a window 100 steps
early from h=0 reproduces the true state to ~1e-5 rel, so the 8 time
shards run independently with a 100-step burn-in (core 0 pads inputs
with zeros, exact). Per core: T=225 steps, B=64 batch.

delta-injection accum-q recurrence (fp16, no per-step leak matmuls, no
per-step DVE z-add). psum holds q = 0.8^-j * p'_t within a Q=32 block
(j = t % Q), where p'_t = z_t + h_{t-1} @ w_eff is the full pre-
activation. Since p'_{t+1} = 0.8 p'_t + r_t @ w_eff + delta_{t+1} with
delta_t = z_t - 0.8 z_{t-1}, each step accumulates into psum:
    8 delta matmuls:  dxT_t @ w_in (4) + identity @ dnT_t (4)
    16 W matmuls:     r'_t @ w_eff
where dxT/dnT are HOST-precomputed deltas, pre-scaled by 0.8^-j(t),
fp16, in transposed layout. Then on DVE:
    r'_t = relu(q * 0.2*0.8^(jp-jn))            (RELU_SC, 1 input)
    h_t  = 0.8*h_{t-1} + 0.8^jn * r'_t          (LEAK2)
Every Q steps the bank is re-injected at true scale via ACT mul
(0.8^Q * q -> fp16) + identity matmuls with start=True.

PE order per step keeps the RA_a chain short: [injA | k01m01] (gated by
r'a) -> k23m01 (gated by r'b, stop A) -> [injC | k01m23 | k23m23]
(bank C in the shadow). RA_b-gated work never sits ahead of RA_a-gated
work in the in-order PE queue.

Output: h16 ring chunks are DMA'd straight to DRAM as fp16 in their
native (p, t, m, b) layout (8-step chunks, round-robin across the three
DMA-capable queues); the host un-transposes to (b, t, r) f32.
"""

import numpy as np
from contextlib import ExitStack

import concourse.bass as bass
import concourse.mybir as mybir
import concourse.tile as tile
from concourse import bacc
from concourse import dve_ops
from concourse.dve_spec import (
    Spec, Src0, Src1, C0, C1, relu as _dve_relu_expr, lower,
)
from concourse.dve_uop import DveOpSpec
from concourse.masks import make_identity


def _register_dve(name, body, ref, rd1=True):
    """Register a custom DVE op (idempotent)."""
    for o in dve_ops.OPS:
        if o.name == name:
            return o
    opcode = max(dve_ops._SUB_OPCODE_FOR_NAME.values()) + 1
    assert opcode < 0x20
    dve_ops._SUB_OPCODE_FOR_NAME[name] = opcode
    spec = Spec(body=body, reference=ref)
    shas = {}
    for ver in ("v3", "v4"):
        s = DveOpSpec(name=name, opcode=opcode, uops=lower(spec, ver=ver),
                      rd1_en=rd1)
        shas[ver] = s.sha(ver)
    op = dve_ops.DveOp(name, spec, subdim=False, uops_sha=shas)
    dve_ops.OPS.append(op)
    dve_ops.CUSTOM_DVE_SPECS[name] = spec
    return op


def _f32(a):
    return a.astype(np.float32).reshape(a.shape[0], -1)


def _ref_relu_sc(in0, in1, c0, c1, c2):
    s = np.maximum(np.nan_to_num(_f32(in0) * c0,
                                 nan=0.0, posinf=np.inf, neginf=-np.inf), 0)
    return s.reshape(in0.shape)


def _ref_leak2(in0, in1, c0, c1, c2):
    return (_f32(in0) * c0 + _f32(in1) * c1).reshape(in0.shape)


RELU_SC = _register_dve("RELU_SC_BIO", _dve_relu_expr(Src0 * C0),
                        _ref_relu_sc, rd1=False)
LEAK2 = _register_dve("LEAK2_BIO", Src0 * C0 + Src1 * C1, _ref_leak2)

F32 = mybir.dt.float32
F16 = mybir.dt.float16
AOP = mybir.AluOpType

R = 512          # n_rec
NIN = 128        # n_in
RC = 4           # r chunks (m and k)
N_CORES = 8
TSPLIT = 8       # time shards
B = 64           # batch per core (full batch)
SUP = RC * B     # cols per step supertile
T_FULL = 1000
T_OUT = T_FULL // TSPLIT  # output steps per core
BURN = 50                 # burn-in steps; truncation + fp16 noise gives
                          # rel err 4.3e-3 vs the 2e-2 gate (validated e2e)
T_LOC = T_OUT + BURN      # local steps per core
OUT0 = BURN               # first local step that produces output
ALPHA = 0.2
LEAK = 1.0 - ALPHA
Q = 32                    # accum-q rescale block
ZR = 128                  # delta ring steps
ZCH = 16                  # delta chunk (DMA granularity)
ZLEAD = 96                # chunks are DMA'd this many steps ahead


def build_nc(T=T_LOC, U=128, use_bacc=True):
    """Build the per-core Bass program. U = h-ring steps."""
    nc = bacc.Bacc() if use_bacc else bass.Bass()

    # host-precomputed pre-scaled deltas, fp16, transposed layouts
    dxT_d = nc.dram_tensor("dxT16", [NIN, T, B], F16, kind="ExternalInput").ap()
    dnT_d = nc.dram_tensor("dnT16", [128, RC, T, B], F16,
                           kind="ExternalInput").ap()
    w_d = nc.dram_tensor("w16", [R, R], F16, kind="ExternalInput").ap()
    wi_d = nc.dram_tensor("win16", [NIN, R], F16, kind="ExternalInput").ap()
    # raw h16 dump: [p, t_out, m*B+b] fp16; host un-transposes
    o_d = nc.dram_tensor("outT16", [128, T_OUT, SUP], F16,
                         kind="ExternalOutput").ap()

    with tile.TileContext(nc) as tc, ExitStack() as ctx:
        const = ctx.enter_context(tc.tile_pool(name="const", bufs=1))
        big = ctx.enter_context(tc.tile_pool(name="big", bufs=1))

        # ---- constants ----
        ident16 = const.tile([128, 128], F16)
        make_identity(nc, ident16[:, :])

        # ---- big persistent buffers ----
        dring = big.tile([128, RC * ZR * B], F16)   # delta-noise ring
        xT16 = big.tile([128, T * B], F16)          # delta-x, full resident
        h16 = big.tile([128, U * SUP], F16)
        nc.vector.memset(h16[:, (U - 1) * SUP:U * SUP], 0.0)

        dv = dring[:, :].rearrange("p (m t b) -> p m t b", t=ZR, b=B)

        # round-robin the bulk DMAs over the three DMA-capable queues
        dmaq = [nc.gpsimd, nc.sync, nc.scalar]
        qi = [0]

        def next_q():
            qi[0] = (qi[0] + 1) % len(dmaq)
            return dmaq[qi[0]]

        def emit_dn_chunk(t0, q=None):
            t1 = min(t0 + ZCH, T)
            for z0 in range(t0, t1, 8):
                nt = min(8, t1 - z0)
                rz = z0 % ZR
                (q or next_q()).dma_start(out=dv[:, :, rz:rz + nt, :],
                                          in_=dnT_d[:, :, z0:z0 + nt, :])

        # startup order: what step 0/1 needs first, spread over queues
        nc.scalar.dma_start(out=xT16[:, :ZCH * B], in_=dxT_d[:, :ZCH, :])
        emit_dn_chunk(0, q=nc.gpsimd)
        win16 = const.tile([128, R], F16)
        nc.scalar.dma_start(out=win16[:, :], in_=wi_d)
        w16 = const.tile([128, RC * R], F16)
        for k in range(RC):
            nc.sync.dma_start(out=w16[:, k * R:(k + 1) * R],
                              in_=w_d[k * 128:(k + 1) * 128, :])
        emit_dn_chunk(ZCH, q=nc.gpsimd)
        # delta-x bulk on the ACT queue
        nc.scalar.dma_start(out=xT16[:, ZCH * B:], in_=dxT_d[:, ZCH:, :])

        # ---- output dump chunks: <=8 steps, never crossing U-multiples;
        # finer near the end so the post-loop tail is small and spread ----
        ochunks = []
        a = OUT0
        while a < T:
            step = 4 if a >= T - 16 else 8
            e = min(a + step, T, ((a // U) + 1) * U)
            ochunks.append((a, e))
            a = e

        def emit_out_chunk(ci):
            a, e = ochunks[ci]
            s0 = (a % U) * SUP
            next_q().dma_start(out=o_d[:, a - OUT0:e - OUT0, :],
                               in_=h16[:, s0:s0 + (e - a) * SUP]
                               .rearrange("p (t s) -> p t s", s=SUP))

        # ---- recurrence ----
        with tc.tile_pool(name="rp", bufs=2) as rp, \
             tc.tile_pool(name="sp", bufs=2) as sp, \
             tc.tile_pool(name="psA", bufs=1, space="PSUM") as ps_a, \
             tc.tile_pool(name="psC", bufs=1, space="PSUM") as ps_c:
            psA = ps_a.tile([128, 512], F32, name="psa", tag="psa")
            psC = ps_c.tile([128, 512], F32, name="psc", tag="psc")
            pvA = psA[:, :2 * B].rearrange("p (m c) -> p m c", c=B)
            pvC = psC[:, :2 * B].rearrange("p (m c) -> p m c", c=B)

            zero16 = const.tile([128, B], F16)
            nc.vector.memset(zero16[:, :], 0.0)

            def ps_of(m):
                ps = psA if m < 2 else psC
                return ps, (m % 2) * B

            def dinj(m, stop=False):
                """delta injections for chunk m: dx@w_in then ident@dn."""
                ps, off = ps_of(m)
                nc.tensor.matmul(
                    ps[:, off:off + B],
                    lhsT=win16[:, m * 128:(m + 1) * 128],
                    rhs=xT16[:, t * B:(t + 1) * B],
                    start=False, stop=False, skip_group_check=True)
                nc.tensor.matmul(
                    ps[:, off:off + B], lhsT=ident16[:, :],
                    rhs=dv[:, m, t % ZR, :],
                    start=False, stop=stop, skip_group_check=True)

            # prime q = 0, then inject delta_0 (= z_0)
            for m in range(RC):
                ps, off = ps_of(m)
                nc.tensor.matmul(ps[:, off:off + B], lhsT=ident16[:, :],
                                 rhs=zero16[:, :], start=(m % 2 == 0),
                                 stop=False, skip_group_check=True)

            for c0 in range(2 * ZCH, ZLEAD + ZCH, ZCH):
                emit_dn_chunk(c0)
            prev_r = None
            for t in range(T):
                if (t + ZLEAD) % ZCH == 0 and ZLEAD + ZCH <= t + ZLEAD < T:
                    emit_dn_chunk(t + ZLEAD)
                rd = ((t - 1) % U) * SUP
                wr = (t % U) * SUP
                rbig = rp.tile([128, SUP], F16, tag="rbig")
                jp = t % Q          # frame of q after this iteration's mms
                jn = (t + 1) % Q    # frame after the next iteration's mms
                if t == 0:
                    for m in range(RC):
                        dinj(m, stop=(m % 2 == 1))
                else:
                    if jp == 0:
                        # restart: re-inject q at true scale (q := 0.8^Q * q)
                        s16a = sp.tile([128, 2 * B], F16, tag="s16a")
                        s16b = sp.tile([128, 2 * B], F16, tag="s16b")
                        nc.scalar.mul(out=s16a[:, :], in_=psA[:, :2 * B],
                                      mul=float(LEAK ** Q))
                        nc.scalar.mul(out=s16b[:, :], in_=psC[:, :2 * B],
                                      mul=float(LEAK ** Q))
                        for m in range(RC):
                            ps, off = ps_of(m)
                            src = s16a if m < 2 else s16b
                            nc.tensor.matmul(
                                ps[:, off:off + B], lhsT=ident16[:, :],
                                rhs=src[:, (m % 2) * B:(m % 2 + 1) * B],
                                start=(m % 2 == 0), stop=False,
                                skip_group_check=True)

                    def kmm(m, k, stop=False):
                        ps, off = ps_of(m)
                        return nc.tensor.matmul(
                            ps[:, off:off + B],
                            lhsT=w16[:, k * R + m * 128:k * R + (m + 1) * 128],
                            rhs=prev_r[:, k * B:(k + 1) * B],
                            start=False, stop=stop, skip_group_check=True)

                    # all r'a-gated work first (never stalls the PE head)
                    dinj(0); dinj(1)
                    kmm(0, 0); kmm(1, 0); kmm(0, 1); kmm(1, 1)
                    kmm(2, 0); kmm(3, 0); kmm(2, 1); kmm(3, 1)
                    # k23m01 (gated by r'b) completes bank A asap
                    kmm(0, 2); kmm(1, 2); kmm(0, 3); kmm(1, 3, stop=True)
                    # bank C tail: injections then k23m23
                    dinj(2); dinj(3)
                    kmm(2, 2); kmm(3, 2); kmm(2, 3); kmm(3, 3, stop=True)

                # r' = relu(q * 0.2*0.8^(jp-jn))   (DVE, psum in only)
                s0 = float(ALPHA * LEAK ** (jp - jn))
                nc.vector._custom_dve(
                    RELU_SC,
                    out=rbig[:, :2 * B].rearrange("p (m c) -> p m c", c=B),
                    in0=pvA[:, 0:2, 0:B], s0=s0)
                nc.vector._custom_dve(
                    RELU_SC,
                    out=rbig[:, 2 * B:].rearrange("p (m c) -> p m c", c=B),
                    in0=pvC[:, 0:2, 0:B], s0=s0)
                # h output: h_t = 0.8*h_{t-1} + 0.8^jn * r'  (off critical path)
                nc.vector._custom_dve(
                    LEAK2,
                    out=h16[:, wr:wr + SUP], in0=h16[:, rd:rd + SUP],
                    in1=rbig[:, :], s0=float(LEAK), s1=float(LEAK ** jn))
                prev_r = rbig
                for ci, (a, e) in enumerate(ochunks):
                    if t == e:
                        emit_out_chunk(ci)
            for ci, (a, e) in enumerate(ochunks):
                if e >= T:
                    emit_out_chunk(ci)

    if use_bacc:
        nc.compile()
    return nc


def host_prep(x, w_in, w_rec, b_rec, ei_mask, autapse_mask, noise):
    """Host-side weight prep + time shard + pre-scaled fp16 delta inputs.

    delta_t = z_t - 0.8*z_{t-1} split into x and noise parts, scaled by
    0.8^-(t % Q) to match the psum accumulation frame. b_rec is folded
    into the noise part (constant offset of z).
    """
    ei = np.diagonal(np.asarray(ei_mask)).astype(np.float32)
    w_eff = ei[:, None] * (np.asarray(w_rec) * np.asarray(autapse_mask))
    w16 = w_eff.astype(np.float16)
    win16 = np.asarray(w_in).astype(np.float16)
    x = np.asarray(x, dtype=np.float32)
    nb = np.asarray(noise, dtype=np.float32) + np.asarray(b_rec, np.float32)
    jscale = (LEAK ** -(np.arange(T_LOC) % Q)).astype(np.float32)
    in_maps = []
    for c in range(N_CORES):
        t0 = c * T_OUT - BURN
        xp = np.zeros((B, T_LOC, NIN), np.float32)
        npad = np.zeros((B, T_LOC, R), np.float32)
        s = max(t0, 0)
        off = s - t0
        xp[:, off:] = x[:, s:t0 + T_LOC]
        npad[:, off:] = nb[:, s:t0 + T_LOC]
        dx = xp.copy()
        dx[:, 1:] -= LEAK * xp[:, :-1]
        dn = npad.copy()
        dn[:, 1:] -= LEAK * npad[:, :-1]
        dx *= jscale[None, :, None]
        dn *= jscale[None, :, None]
        dxT = np.ascontiguousarray(
            dx.astype(np.float16).transpose(2, 1, 0))
        dnT = np.ascontiguousarray(
            dn.astype(np.float16).reshape(B, T_LOC, RC, 128)
            .transpose(3, 2, 1, 0))
        in_maps.append({
            "dxT16": dxT,
            "dnT16": dnT,
            "w16": w16,
            "win16": win16,
        })
    return in_maps, w_eff.astype(np.float32)


def reference_np(x, w_in, b_rec, w_eff, noise, T=None):
    """Numpy reference for dev checks (f32)."""
    x = np.asarray(x, np.float32)
    if T is None:
        T = x.shape[1]
    z = np.einsum("bti,ir->btr", x[:, :T], np.asarray(w_in)) \
        + np.asarray(noise)[:, :T] + np.asarray(b_rec)
    h = np.zeros((x.shape[0], w_eff.shape[0]), np.float32)
    outs = []
    for t in range(T):
        pre = z[:, t] + h @ w_eff
        h = LEAK * h + ALPHA * np.maximum(pre, 0.0)
        outs.append(h.copy())
    return np.stack(outs, axis=1)


# ---------------------------------------------------------------------------
# harness entry point
# ---------------------------------------------------------------------------
_NC_CACHE = {}


def kernel(x, w_in, w_rec, b_rec, ei_mask, autapse_mask, noise):
    from concourse.bass_utils import run_bass_kernel_spmd

    x = np.asarray(x)
    T = x.shape[1]
    in_maps, _ = host_prep(x, w_in, w_rec, b_rec, ei_mask, autapse_mask, noise)
    if T not in _NC_CACHE:
        _NC_CACHE[T] = build_nc()
    nc = _NC_CACHE[T]
    res = run_bass_kernel_spmd(nc, in_maps, core_ids=list(range(N_CORES)))
    out = np.empty((x.shape[0], T, R), np.float32)
    for c in range(N_CORES):
        # dump[p, t, m*B+b] = h[b, t, m*128+p]
        dump = res.results[c]["outT16"]
        out[:, c * T_OUT:(c + 1) * T_OUT] = (
            dump.reshape(128, T_OUT, RC, B).transpose(3, 1, 2, 0)
            .reshape(B, T_OUT, R).astype(np.float32))
    return out



# revision 5
# speedup vs baseline: 1.0903x; 1.0903x over previous
"""BioRNN Trainium2 kernel (dev module).

Sharding: 16 time-windows (2 per core, ~63 output steps each), full
batch 64 per window. The leak (0.8/step) forgets initial state: a
32-step burn-in from h=0 reproduces the true state (window 0 pads with
zeros, exact). Per core: 2 windows x T_LOC=95 steps, emitted half-step
out of phase so one window's relu handoff hides under the other's
matmuls.

accum-q recurrence, v-part only in psum: q = 0.8^-j * v_t within a
Q=32 block (j = t % Q), v_t = h_{t-1} @ w_eff. Per window-step:
    1 inj matmul   identity @ dz_t (bank A only; N=128)
    16 W matmuls   r'_t @ w_eff (N=64)
where dz is the HOST-precomputed delta of z = x@w_in + noise + b_rec
for the bank-A half (r 0:256), pre-scaled by 0.8^-j, fp16, transposed.
Bank C's z half never enters psum: it rides in through a 2-input DVE
relu. The two relus run on DIFFERENT engines to halve the handoff
serialization:
    bank A (r 0:256):  ACT   r'a = relu(qA * c0)         (z in psum)
    bank C (r 256:512): DVE  r'c = relu(qC*c0 + z*c0)    (z from SBUF)
with c0 = 0.2*0.8^(jp-jn). Every Q steps each bank is re-injected at
true scale (ACT/DVE mul -> fp16 -> identity matmul, start=True).

h is NOT computed on device: h_t = 0.8 h_{t-1} + 0.8^jn r'_t is a
leaky integration the HOST does in f32 over the dumped r' stream
(more accurate than the old fp16 device ring). r' ring chunks DMA to
DRAM in native (p, t, v, m, b) fp16 layout; host un-transposes,
rescales by 0.8^jn(t), and lfilters to h.
"""

import os
import numpy as np
from contextlib import ExitStack

import concourse.bass as bass
import concourse.mybir as mybir
import concourse.tile as tile
from concourse import bacc
from concourse import dve_ops
from concourse.dve_spec import (
    Spec, Src0, Src1, C0, C1, relu as _dve_relu_expr, lower,
)
from concourse.dve_uop import DveOpSpec
from concourse.masks import make_identity


def _register_dve(name, body, ref, rd1=True):
    """Register a custom DVE op (idempotent)."""
    for o in dve_ops.OPS:
        if o.name == name:
            return o
    opcode = max(dve_ops._SUB_OPCODE_FOR_NAME.values()) + 1
    assert opcode < 0x20
    dve_ops._SUB_OPCODE_FOR_NAME[name] = opcode
    spec = Spec(body=body, reference=ref)
    shas = {}
    for ver in ("v3", "v4"):
        s = DveOpSpec(name=name, opcode=opcode, uops=lower(spec, ver=ver),
                      rd1_en=rd1)
        shas[ver] = s.sha(ver)
    op = dve_ops.DveOp(name, spec, subdim=False, uops_sha=shas)
    dve_ops.OPS.append(op)
    dve_ops.CUSTOM_DVE_SPECS[name] = spec
    return op


def _f32(a):
    return a.astype(np.float32).reshape(a.shape[0], -1)


def _ref_relu2(in0, in1, c0, c1, c2):
    s = np.maximum(np.nan_to_num(_f32(in0) * c0 + _f32(in1) * c1,
                                 nan=0.0, posinf=np.inf, neginf=-np.inf), 0)
    return s.reshape(in0.shape)


RELU2 = _register_dve("RELU2_BIO", _dve_relu_expr(Src0 * C0 + Src1 * C1),
                      _ref_relu2, rd1=True)

F32 = mybir.dt.float32
F16 = mybir.dt.float16
RELU_FN = mybir.ActivationFunctionType.Relu

R = 512          # n_rec
NIN = 128        # n_in
RC = 4           # r chunks
B = 64           # batch (full, per window)
N_CORES = 8
NV = 2           # windows per core
NWIN = N_CORES * NV
T_FULL = 1000
BURN = int(os.environ.get("BIO_BURN", "48"))  # burn-in steps
WSTARTS = [(w * T_FULL) // NWIN for w in range(NWIN + 1)]
WLEN = max(WSTARTS[w + 1] - WSTARTS[w] for w in range(NWIN))  # 63
T_LOC = WLEN + BURN       # local steps per window
ALPHA = 0.2
LEAK = 1.0 - ALPHA
Q = 32                    # accum-q rescale block
U = 32                    # r' ring steps
CPS = NV * 256            # ring cols per step-slot
ZCH = 8                   # z-stream DMA chunk (steps)
ZLEAD = 24                # z chunks DMA'd this many steps ahead


def build_nc(T=T_LOC):
    """Build the per-core Bass program."""
    nc = bacc.Bacc()

    # host-precomputed pre-scaled z streams, fp16, transposed layout
    # [p, v, t, c]: c = m*64+b; c 0:128 = delta-enc (bank A), 128:256 = raw
    z_d = nc.dram_tensor("z16", [128, NV, T, 256], F16,
                         kind="ExternalInput").ap()
    w_d = nc.dram_tensor("w16", [R, R], F16, kind="ExternalInput").ap()
    # raw r' dump: [p, t, v*256 + m*64 + b] fp16; host integrates h
    o_d = nc.dram_tensor("outT16", [128, T, CPS], F16,
                         kind="ExternalOutput").ap()

    with tile.TileContext(nc) as tc, ExitStack() as ctx:
        const = ctx.enter_context(tc.tile_pool(name="const", bufs=1))
        big = ctx.enter_context(tc.tile_pool(name="big", bufs=1))
        sp = ctx.enter_context(tc.tile_pool(name="sp", bufs=2))

        ident16 = const.tile([128, 128], F16)
        make_identity(nc, ident16[:, :])
        zero16 = const.tile([128, 128], F16)
        nc.vector.memset(zero16[:, :], 0.0)

        w16 = big.tile([128, RC * R], F16)
        zt = big.tile([128, NV * T * 256], F16)   # z streams, full resident
        rr = big.tile([128, U * CPS], F16)        # r' ring

        pools = [ctx.enter_context(
            tc.tile_pool(name=f"ps{n}", bufs=1, space="PSUM"))
            for n in ("a0", "c0", "a1", "c1")]
        pst = [p.tile([128, 512], F32, name=f"ps{i}", tag=f"ps{i}")
               for i, p in enumerate(pools)]
        psA = [pst[0], pst[2]]
        psC = [pst[1], pst[3]]

        # ---- DMA helpers ----
        qin = [nc.gpsimd, nc.sync]
        qout = [nc.scalar, nc.gpsimd]
        qi = [0, 0]

        def emit_in(v, t0):
            t1 = min(t0 + ZCH, T)
            q = qin[qi[0] % 2]
            qi[0] += 1
            q.dma_start(
                out=zt[:, (v * T + t0) * 256:(v * T + t1) * 256]
                .rearrange("p (t c) -> p t c", c=256),
                in_=z_d[:, v, t0:t1, :])

        def emit_out(a, e):
            q = qout[qi[1] % 2]
            qi[1] += 1
            s0 = (a % U) * CPS
            q.dma_start(
                out=o_d[:, a:e, :],
                in_=rr[:, s0:s0 + (e - a) * CPS]
                .rearrange("p (t c) -> p t c", c=CPS))

        # startup: first z chunks, then weights
        for t0 in range(0, min(ZLEAD, T), ZCH):
            emit_in(0, t0)
            emit_in(1, t0)
        for k in range(RC):
            nc.sync.dma_start(out=w16[:, k * R:(k + 1) * R],
                              in_=w_d[k * 128:(k + 1) * 128, :])

        mm = nc.tensor.matmul

        def block(v, t):
            pa, pc = psA[v], psC[v]
            base = (v * T + t) * 256
            zA = zt[:, base:base + 128]
            zC = zt[:, base + 128:base + 256]
            so = (t % U) * CPS + v * 256
            outA = rr[:, so:so + 128]
            outC = rr[:, so + 128:so + 256]
            pr = ((t - 1) % U) * CPS + v * 256

            def rk(k):
                return rr[:, pr + k * 64:pr + (k + 1) * 64]

            if t == 0:
                mm(pa[:, :128], lhsT=ident16[:, :], rhs=zA,
                   start=True, stop=True, skip_group_check=True)
                mm(pc[:, :128], lhsT=ident16[:, :], rhs=zero16[:, :],
                   start=True, stop=True, skip_group_check=True)
            else:
                if t % Q == 0:
                    # re-inject q at true scale (q := 0.8^Q * q)
                    s16a = sp.tile([128, 128], F16, tag=f"s16a{v}")
                    s16c = sp.tile([128, 128], F16, tag=f"s16c{v}")
                    nc.scalar.mul(out=s16a[:, :], in_=pa[:, :128],
                                  mul=float(LEAK ** Q))
                    nc.vector.tensor_scalar_mul(s16c[:, :], pc[:, :128],
                                                float(LEAK ** Q))
                    mm(pa[:, :128], lhsT=ident16[:, :], rhs=s16a[:, :],
                       start=True, stop=False, skip_group_check=True)
                    mm(pc[:, :128], lhsT=ident16[:, :], rhs=s16c[:, :],
                       start=True, stop=False, skip_group_check=True)

                def kmm(m, k, stop=False):
                    ps = pa if m < 2 else pc
                    off = (m % 2) * 64
                    mm(ps[:, off:off + 64],
                       lhsT=w16[:, k * R + m * 128:k * R + (m + 1) * 128],
                       rhs=rk(k), start=False, stop=stop,
                       skip_group_check=True)

                mm(pa[:, :128], lhsT=ident16[:, :], rhs=zA,
                   start=False, stop=False, skip_group_check=True)
                kmm(0, 0); kmm(1, 0); kmm(0, 1); kmm(1, 1)
                kmm(2, 0); kmm(3, 0); kmm(2, 1); kmm(3, 1)
                kmm(0, 2); kmm(1, 2); kmm(0, 3); kmm(1, 3, stop=True)
                kmm(2, 2); kmm(3, 2); kmm(2, 3); kmm(3, 3, stop=True)

            jp = t % Q
            jn = (t + 1) % Q
            c0 = float(ALPHA * LEAK ** (jp - jn))
            nc.scalar.activation(out=outA, in_=pa[:, :128], func=RELU_FN,
                                 scale=c0)
            nc.vector._custom_dve(RELU2, out=outC, in0=pc[:, :128],
                                  in1=zC, s0=c0, s1=c0)

        for t in range(T):
            tp = t + ZLEAD
            if tp % ZCH == 0 and tp < T:
                emit_in(0, tp)
                emit_in(1, tp)
            block(0, t)
            block(1, t)
            if (t + 1) % ZCH == 0 or t == T - 1:
                a = (t // ZCH) * ZCH
                emit_out(a, t + 1)

    nc.compile()
    return nc


def host_prep(x, w_in, w_rec, b_rec, ei_mask, autapse_mask, noise):
    """Host-side weight prep + window shard + pre-scaled fp16 z streams.

    z = x@w_in + noise + b_rec. Bank-A half (r 0:256): delta-encoded
    (z_t - 0.8 z_{t-1}) for psum injection. Bank-C half (r 256:512):
    raw (added at the DVE relu). Both scaled by 0.8^-(t % Q).
    """
    ei = np.diagonal(np.asarray(ei_mask)).astype(np.float32)
    w_eff = ei[:, None] * (np.asarray(w_rec) * np.asarray(autapse_mask))
    w16 = w_eff.astype(np.float16)
    x = np.asarray(x, dtype=np.float32)
    z = (x.reshape(-1, NIN) @ np.asarray(w_in, dtype=np.float32)).reshape(
        B, T_FULL, R)
    z += np.asarray(noise, dtype=np.float32)
    z += np.asarray(b_rec, np.float32)
    jscale = (LEAK ** -(np.arange(T_LOC) % Q)).astype(np.float32)
    in_maps = []
    for c in range(N_CORES):
        zwins = []
        for v in range(NV):
            w = NV * c + v
            t0 = WSTARTS[w] - BURN
            zp = np.zeros((B, T_LOC, R), np.float32)
            s = max(t0, 0)
            zp[:, s - t0:] = z[:, s:t0 + T_LOC]
            # [p, t, m, b]
            zt4 = zp.reshape(B, T_LOC, RC, 128).transpose(3, 1, 2, 0)
            dA = zt4[:, :, 0:2, :].copy()
            dA[:, 1:] -= LEAK * dA[:, :-1].copy()
            dA *= jscale[None, :, None, None]
            zC = zt4[:, :, 2:4, :] * jscale[None, :, None, None]
            zwins.append(np.concatenate(
                [dA.reshape(128, T_LOC, 128), zC.reshape(128, T_LOC, 128)],
                axis=2))
        z16 = np.ascontiguousarray(
            np.stack(zwins, axis=1).astype(np.float16))
        in_maps.append({"z16": z16, "w16": w16})
    return in_maps, w_eff.astype(np.float32)


def _integrate(dump):
    """dump: [128, T, CPS] fp16 per core -> list of NV h arrays
    (B, T_LOC, R) f32 via host leaky integration."""
    jn = ((np.arange(T_LOC) + 1) % Q).astype(np.float32)
    sc = (LEAK ** jn).astype(np.float32)
    hs = []
    for v in range(NV):
        rp = dump[:, :, v * 256:(v + 1) * 256].astype(np.float32)
        # [p, t, m, b] -> [b, t, r]
        ar = rp.reshape(128, T_LOC, RC, B).transpose(3, 1, 2, 0).reshape(
            B, T_LOC, R)
        ar *= sc[None, :, None]
        h = np.empty_like(ar)
        acc = np.zeros((B, R), np.float32)
        for t in range(T_LOC):
            acc = LEAK * acc + ar[:, t]
            h[:, t] = acc
        hs.append(h)
    return hs


def reference_np(x, w_in, b_rec, w_eff, noise, T=None):
    """Numpy reference for dev checks (f32)."""
    x = np.asarray(x, np.float32)
    if T is None:
        T = x.shape[1]
    z = np.einsum("bti,ir->btr", x[:, :T], np.asarray(w_in)) \
        + np.asarray(noise)[:, :T] + np.asarray(b_rec)
    h = np.zeros((x.shape[0], w_eff.shape[0]), np.float32)
    outs = []
    for t in range(T):
        pre = z[:, t] + h @ w_eff
        h = LEAK * h + ALPHA * np.maximum(pre, 0.0)
        outs.append(h.copy())
    return np.stack(outs, axis=1)


# ---------------------------------------------------------------------------
# harness entry point
# ---------------------------------------------------------------------------
_NC_CACHE = {}


def kernel(x, w_in, w_rec, b_rec, ei_mask, autapse_mask, noise):
    from concourse.bass_utils import run_bass_kernel_spmd

    x = np.asarray(x)
    T = x.shape[1]
    in_maps, _ = host_prep(x, w_in, w_rec, b_rec, ei_mask, autapse_mask, noise)
    if T not in _NC_CACHE:
        _NC_CACHE[T] = build_nc()
    nc = _NC_CACHE[T]
    res = run_bass_kernel_spmd(nc, in_maps, core_ids=list(range(N_CORES)))
    out = np.empty((x.shape[0], T, R), np.float32)
    for c in range(N_CORES):
        hs = _integrate(res.results[c]["outT16"])
        for v in range(NV):
            w = NV * c + v
            a, e = WSTARTS[w], WSTARTS[w + 1]
            out[:, a:e] = hs[v][:, BURN:BURN + (e - a)]
    return out


# revision 12
# speedup vs baseline: 1.0979x; 1.0070x over previous
"""BioRNN Trainium2 kernel (dev module).

Sharding: 16 time-windows (2 per core, ~63 output steps each), full
batch 64 per window. The leak (0.8/step) forgets initial state: a
32-step burn-in from h=0 reproduces the true state (window 0 pads with
zeros, exact). Per core: 2 windows x T_LOC=95 steps, emitted half-step
out of phase so one window's relu handoff hides under the other's
matmuls.

accum-q recurrence, v-part only in psum: q = 0.8^-j * v_t within a
Q=32 block (j = t % Q), v_t = h_{t-1} @ w_eff. Per window-step:
    1 inj matmul   identity @ dz_t (bank A only; N=128)
    16 W matmuls   r'_t @ w_eff (N=64)
where dz is the HOST-precomputed delta of z = x@w_in + noise + b_rec
for the bank-A half (r 0:256), pre-scaled by 0.8^-j, fp16, transposed.
Bank C's z half never enters psum: it rides in through a 2-input DVE
relu. The two relus run on DIFFERENT engines to halve the handoff
serialization:
    bank A (r 0:256):  ACT   r'a = relu(qA * c0)         (z in psum)
    bank C (r 256:512): DVE  r'c = relu(qC*c0 + z*c0)    (z from SBUF)
with c0 = 0.2*0.8^(jp-jn). Every Q steps each bank is re-injected at
true scale (ACT/DVE mul -> fp16 -> identity matmul, start=True).

h is NOT computed on device: h_t = 0.8 h_{t-1} + 0.8^jn r'_t is a
leaky integration the HOST does in f32 over the dumped r' stream
(more accurate than the old fp16 device ring). r' ring chunks DMA to
DRAM in native (p, t, v, m, b) fp16 layout; host un-transposes,
rescales by 0.8^jn(t), and lfilters to h.
"""

import os
import numpy as np
from contextlib import ExitStack

import concourse.bass as bass
import concourse.mybir as mybir
import concourse.tile as tile
from concourse import bacc
from concourse import dve_ops
from concourse.dve_spec import (
    Spec, Src0, Src1, C0, C1, relu as _dve_relu_expr, lower,
)
from concourse.dve_uop import DveOpSpec
from concourse.masks import make_identity


def _register_dve(name, body, ref, rd1=True):
    """Register a custom DVE op (idempotent)."""
    for o in dve_ops.OPS:
        if o.name == name:
            return o
    opcode = max(dve_ops._SUB_OPCODE_FOR_NAME.values()) + 1
    assert opcode < 0x20
    dve_ops._SUB_OPCODE_FOR_NAME[name] = opcode
    spec = Spec(body=body, reference=ref)
    shas = {}
    for ver in ("v3", "v4"):
        s = DveOpSpec(name=name, opcode=opcode, uops=lower(spec, ver=ver),
                      rd1_en=rd1)
        shas[ver] = s.sha(ver)
    op = dve_ops.DveOp(name, spec, subdim=False, uops_sha=shas)
    dve_ops.OPS.append(op)
    dve_ops.CUSTOM_DVE_SPECS[name] = spec
    return op


def _f32(a):
    return a.astype(np.float32).reshape(a.shape[0], -1)


def _ref_relu2(in0, in1, c0, c1, c2):
    s = np.maximum(np.nan_to_num(_f32(in0) * c0 + _f32(in1) * c1,
                                 nan=0.0, posinf=np.inf, neginf=-np.inf), 0)
    return s.reshape(in0.shape)


RELU2 = _register_dve("RELU2_BIO", _dve_relu_expr(Src0 * C0 + Src1 * C1),
                      _ref_relu2, rd1=True)

F32 = mybir.dt.float32
F16 = mybir.dt.float16
RELU_FN = mybir.ActivationFunctionType.Relu

R = 512          # n_rec
NIN = 128        # n_in
RC = 4           # r chunks
B = 64           # batch (full, per window)
N_CORES = 8
NV = 2           # windows per core
NWIN = N_CORES * NV
T_FULL = 1000
BURN = int(os.environ.get("BIO_BURN", "48"))  # burn-in steps
WSTARTS = [(w * T_FULL) // NWIN for w in range(NWIN + 1)]
WLEN = max(WSTARTS[w + 1] - WSTARTS[w] for w in range(NWIN))  # 63
T_LOC = WLEN + BURN       # local steps per window
ALPHA = 0.2
LEAK = 1.0 - ALPHA
Q = 40                    # accum-q rescale block (0.8^-39 ~ 6e3: fp16-safe)
QOFF = (0, Q // 2)        # per-window frame offset: restarts never coincide
U = 48                    # r' ring steps
CPS = NV * 256            # ring cols per step-slot
ZCH = 8                   # z-stream DMA chunk (steps)
ZLEAD = 40                # z chunks DMA'd this many steps ahead
OCH = 4                   # out-dump chunk (steps)


def build_nc(T=T_LOC):
    """Build the per-core Bass program."""
    nc = bacc.Bacc()

    # host-precomputed pre-scaled z streams, fp16, transposed layout
    # [p, v, t, c]: c = m*64+b; c 0:128 = delta-enc (bank A), 128:256 = raw
    z_d = nc.dram_tensor("z16", [128, NV, T, 256], F16,
                         kind="ExternalInput").ap()
    w_d = nc.dram_tensor("w16", [R, R], F16, kind="ExternalInput").ap()
    # raw r' dump: [p, t, v*256 + m*64 + b] fp16; host integrates h
    o_d = nc.dram_tensor("outT16", [128, T, CPS], F16,
                         kind="ExternalOutput").ap()

    with tile.TileContext(nc) as tc, ExitStack() as ctx:
        const = ctx.enter_context(tc.tile_pool(name="const", bufs=1))
        big = ctx.enter_context(tc.tile_pool(name="big", bufs=1))
        sp = ctx.enter_context(tc.tile_pool(name="sp", bufs=2))

        ident16 = const.tile([128, 128], F16)
        make_identity(nc, ident16[:, :])
        zero16 = const.tile([128, 128], F16)
        nc.vector.memset(zero16[:, :], 0.0)

        w16 = big.tile([128, RC * R], F16)
        zt = big.tile([128, NV * T * 256], F16)   # z streams, full resident
        rr = big.tile([128, U * CPS], F16)        # r' ring

        pools = [ctx.enter_context(
            tc.tile_pool(name=f"ps{n}", bufs=1, space="PSUM"))
            for n in ("a0", "c0", "a1", "c1")]
        pst = [p.tile([128, 512], F32, name=f"ps{i}", tag=f"ps{i}")
               for i, p in enumerate(pools)]
        psA = [pst[0], pst[2]]
        psC = [pst[1], pst[3]]

        # ---- DMA helpers: dedicated queues per stream ----
        qin = [nc.gpsimd, nc.sync]

        def emit_in(v, t0, t1):
            t1 = min(t1, T)
            if t0 >= t1:
                return
            qin[v].dma_start(
                out=zt[:, (v * T + t0) * 256:(v * T + t1) * 256]
                .rearrange("p (t c) -> p t c", c=256),
                in_=z_d[:, v, t0:t1, :])

        def emit_out(a, e):
            s0 = (a % U) * CPS
            nc.scalar.dma_start(
                out=o_d[:, a:e, :],
                in_=rr[:, s0:s0 + (e - a) * CPS]
                .rearrange("p (t c) -> p t c", c=CPS))

        # startup: step-0 slivers first, then weights, then the lead chunks
        emit_in(0, 0, 1)
        emit_in(1, 0, 1)
        for k in range(RC):
            (nc.sync if k % 2 else nc.gpsimd).dma_start(
                out=w16[:, k * R:(k + 1) * R],
                in_=w_d[k * 128:(k + 1) * 128, :])
        for v in range(NV):
            emit_in(v, 1, 8)
        for t0 in range(ZCH, min(ZLEAD, T), ZCH):
            for v in range(NV):
                emit_in(v, t0, t0 + ZCH)

        mm = nc.tensor.matmul

        def block(v, t):
            pa, pc = psA[v], psC[v]
            base = (v * T + t) * 256
            zA = zt[:, base:base + 128]
            zC = zt[:, base + 128:base + 256]
            so = (t % U) * CPS + v * 256
            outA = rr[:, so:so + 128]
            outC = rr[:, so + 128:so + 256]
            pr = ((t - 1) % U) * CPS + v * 256

            def rk(k):
                return rr[:, pr + k * 64:pr + (k + 1) * 64]

            if t == 0:
                mm(pa[:, :128], lhsT=ident16[:, :], rhs=zA,
                   start=True, stop=True, skip_group_check=True)
                mm(pc[:, :128], lhsT=ident16[:, :], rhs=zero16[:, :],
                   start=True, stop=True, skip_group_check=True)
            else:
                if (t + QOFF[v]) % Q == 0:
                    # re-inject q at true scale (q := 0.8^Q * q)
                    s16a = sp.tile([128, 128], F16, tag=f"s16a{v}")
                    s16c = sp.tile([128, 128], F16, tag=f"s16c{v}")
                    nc.scalar.mul(out=s16a[:, :], in_=pa[:, :128],
                                  mul=float(LEAK ** Q))
                    nc.vector.tensor_scalar_mul(s16c[:, :], pc[:, :128],
                                                float(LEAK ** Q))
                    mm(pa[:, :128], lhsT=ident16[:, :], rhs=s16a[:, :],
                       start=True, stop=False, skip_group_check=True)
                    mm(pc[:, :128], lhsT=ident16[:, :], rhs=s16c[:, :],
                       start=True, stop=False, skip_group_check=True)

                def kmm(m, k, stop=False):
                    ps = pa if m < 2 else pc
                    off = (m % 2) * 64
                    mm(ps[:, off:off + 64],
                       lhsT=w16[:, k * R + m * 128:k * R + (m + 1) * 128],
                       rhs=rk(k), start=False, stop=stop,
                       skip_group_check=True)

                mm(pa[:, :128], lhsT=ident16[:, :], rhs=zA,
                   start=False, stop=False, skip_group_check=True)
                kmm(0, 0); kmm(1, 0); kmm(0, 1); kmm(1, 1)
                kmm(2, 0); kmm(3, 0); kmm(2, 1); kmm(3, 1)
                kmm(0, 2); kmm(1, 2); kmm(0, 3); kmm(1, 3, stop=True)
                kmm(2, 2); kmm(3, 2); kmm(2, 3); kmm(3, 3, stop=True)

            jp = (t + QOFF[v]) % Q
            jn = (t + 1 + QOFF[v]) % Q
            c0 = float(ALPHA * LEAK ** (jp - jn))
            nc.scalar.activation(out=outA, in_=pa[:, :128], func=RELU_FN,
                                 scale=c0)
            nc.vector._custom_dve(RELU2, out=outC, in0=pc[:, :128],
                                  in1=zC, s0=c0, s1=c0)

        for t in range(T):
            tp = t + ZLEAD
            if tp % ZCH == 0 and tp < T:
                emit_in(0, tp, tp + ZCH)
                emit_in(1, tp, tp + ZCH)
            block(0, t)
            block(1, t)
            if (t + 1) % OCH == 0 or t == T - 1:
                a = (t // OCH) * OCH
                emit_out(a, t + 1)

    nc.compile()
    return nc


def host_prep(x, w_in, w_rec, b_rec, ei_mask, autapse_mask, noise):
    """Host-side weight prep + window shard + pre-scaled fp16 z streams.

    z = x@w_in + noise + b_rec. Bank-A half (r 0:256): delta-encoded
    (z_t - 0.8 z_{t-1}) for psum injection. Bank-C half (r 256:512):
    raw (added at the DVE relu). Both scaled by 0.8^-(t % Q).
    """
    ei = np.diagonal(np.asarray(ei_mask)).astype(np.float32)
    w_eff = ei[:, None] * (np.asarray(w_rec) * np.asarray(autapse_mask))
    w16 = w_eff.astype(np.float16)
    x = np.asarray(x, dtype=np.float32)
    z = (x.reshape(-1, NIN) @ np.asarray(w_in, dtype=np.float32)).reshape(
        B, T_FULL, R)
    z += np.asarray(noise, dtype=np.float32)
    z += np.asarray(b_rec, np.float32)
    in_maps = []
    for c in range(N_CORES):
        zwins = []
        for v in range(NV):
            jscale = (LEAK ** -((np.arange(T_LOC) + QOFF[v]) % Q)
                      ).astype(np.float32)
            w = NV * c + v
            t0 = WSTARTS[w] - BURN
            zp = np.zeros((B, T_LOC, R), np.float32)
            s = max(t0, 0)
            zp[:, s - t0:] = z[:, s:t0 + T_LOC]
            # [p, t, m, b]
            zt4 = zp.reshape(B, T_LOC, RC, 128).transpose(3, 1, 2, 0)
            dA = zt4[:, :, 0:2, :].copy()
            dA[:, 1:] -= LEAK * dA[:, :-1].copy()
            dA *= jscale[None, :, None, None]
            zC = zt4[:, :, 2:4, :] * jscale[None, :, None, None]
            zwins.append(np.concatenate(
                [dA.reshape(128, T_LOC, 128), zC.reshape(128, T_LOC, 128)],
                axis=2))
        z16 = np.ascontiguousarray(
            np.stack(zwins, axis=1).astype(np.float16))
        in_maps.append({"z16": z16, "w16": w16})
    return in_maps, w_eff.astype(np.float32)


def _integrate(dump):
    """dump: [128, T, CPS] fp16 per core -> list of NV h arrays
    (B, T_LOC, R) f32 via host leaky integration."""
    hs = []
    for v in range(NV):
        jn = ((np.arange(T_LOC) + 1 + QOFF[v]) % Q).astype(np.float32)
        sc = (LEAK ** jn).astype(np.float32)
        rp = dump[:, :, v * 256:(v + 1) * 256].astype(np.float32)
        # [p, t, m, b] -> [b, t, r]
        ar = rp.reshape(128, T_LOC, RC, B).transpose(3, 1, 2, 0).reshape(
            B, T_LOC, R)
        ar *= sc[None, :, None]
        h = np.empty_like(ar)
        acc = np.zeros((B, R), np.float32)
        for t in range(T_LOC):
            acc = LEAK * acc + ar[:, t]
            h[:, t] = acc
        hs.append(h)
    return hs


def reference_np(x, w_in, b_rec, w_eff, noise, T=None):
    """Numpy reference for dev checks (f32)."""
    x = np.asarray(x, np.float32)
    if T is None:
        T = x.shape[1]
    z = np.einsum("bti,ir->btr", x[:, :T], np.asarray(w_in)) \
        + np.asarray(noise)[:, :T] + np.asarray(b_rec)
    h = np.zeros((x.shape[0], w_eff.shape[0]), np.float32)
    outs = []
    for t in range(T):
        pre = z[:, t] + h @ w_eff
        h = LEAK * h + ALPHA * np.maximum(pre, 0.0)
        outs.append(h.copy())
    return np.stack(outs, axis=1)


# ---------------------------------------------------------------------------
# harness entry point
# ---------------------------------------------------------------------------
_NC_CACHE = {}


def kernel(x, w_in, w_rec, b_rec, ei_mask, autapse_mask, noise):
    from concourse.bass_utils import run_bass_kernel_spmd

    x = np.asarray(x)
    T = x.shape[1]
    in_maps, _ = host_prep(x, w_in, w_rec, b_rec, ei_mask, autapse_mask, noise)
    if T not in _NC_CACHE:
        _NC_CACHE[T] = build_nc()
    nc = _NC_CACHE[T]
    res = run_bass_kernel_spmd(nc, in_maps, core_ids=list(range(N_CORES)))
    out = np.empty((x.shape[0], T, R), np.float32)
    for c in range(N_CORES):
        hs = _integrate(res.results[c]["outT16"])
        for v in range(NV):
            w = NV * c + v
            a, e = WSTARTS[w], WSTARTS[w + 1]
            out[:, a:e] = hs[v][:, BURN:BURN + (e - a)]
    return out


# revision 13
# speedup vs baseline: 1.3086x; 1.1919x over previous
"""BioRNN Trainium2 kernel (dev module).

Sharding: 16 time-windows (2 per core, ~63 output steps each), full
batch 64 per window. The leak (0.8/step) forgets initial state: a
32-step burn-in from h=0 reproduces the true state (window 0 pads with
zeros, exact). Per core: 2 windows x T_LOC=95 steps, emitted half-step
out of phase so one window's relu handoff hides under the other's
matmuls.

accum-q recurrence, v-part only in psum: q = 0.8^-j * v_t within a
Q=32 block (j = t % Q), v_t = h_{t-1} @ w_eff. Per window-step:
    1 inj matmul   identity @ dz_t (bank A only; N=128)
    16 W matmuls   r'_t @ w_eff (N=64)
where dz is the HOST-precomputed delta of z = x@w_in + noise + b_rec
for the bank-A half (r 0:256), pre-scaled by 0.8^-j, fp16, transposed.
Bank C's z half never enters psum: it rides in through a 2-input DVE
relu. The two relus run on DIFFERENT engines to halve the handoff
serialization:
    bank A (r 0:256):  ACT   r'a = relu(qA * c0)         (z in psum)
    bank C (r 256:512): DVE  r'c = relu(qC*c0 + z*c0)    (z from SBUF)
with c0 = 0.2*0.8^(jp-jn). Every Q steps each bank is re-injected at
true scale (ACT/DVE mul -> fp16 -> identity matmul, start=True).

h is NOT computed on device: h_t = 0.8 h_{t-1} + 0.8^jn r'_t is a
leaky integration the HOST does in f32 over the dumped r' stream
(more accurate than the old fp16 device ring). r' ring chunks DMA to
DRAM in native (p, t, v, m, b) fp16 layout; host un-transposes,
rescales by 0.8^jn(t), and lfilters to h.
"""

import os
import numpy as np
from contextlib import ExitStack

import concourse.bass as bass
import concourse.mybir as mybir
import concourse.tile as tile
from concourse import bacc
from concourse import dve_ops
from concourse.dve_spec import (
    Spec, Src0, Src1, C0, C1, relu as _dve_relu_expr, lower,
)
from concourse.dve_uop import DveOpSpec
from concourse.masks import make_identity


def _register_dve(name, body, ref, rd1=True):
    """Register a custom DVE op (idempotent)."""
    for o in dve_ops.OPS:
        if o.name == name:
            return o
    opcode = max(dve_ops._SUB_OPCODE_FOR_NAME.values()) + 1
    assert opcode < 0x20
    dve_ops._SUB_OPCODE_FOR_NAME[name] = opcode
    spec = Spec(body=body, reference=ref)
    shas = {}
    for ver in ("v3", "v4"):
        s = DveOpSpec(name=name, opcode=opcode, uops=lower(spec, ver=ver),
                      rd1_en=rd1)
        shas[ver] = s.sha(ver)
    op = dve_ops.DveOp(name, spec, subdim=False, uops_sha=shas)
    dve_ops.OPS.append(op)
    dve_ops.CUSTOM_DVE_SPECS[name] = spec
    return op


def _f32(a):
    return a.astype(np.float32).reshape(a.shape[0], -1)


def _ref_relu2(in0, in1, c0, c1, c2):
    s = np.maximum(np.nan_to_num(_f32(in0) * c0 + _f32(in1) * c1,
                                 nan=0.0, posinf=np.inf, neginf=-np.inf), 0)
    return s.reshape(in0.shape)


RELU2 = _register_dve("RELU2_BIO", _dve_relu_expr(Src0 * C0 + Src1 * C1),
                      _ref_relu2, rd1=True)

F32 = mybir.dt.float32
F16 = mybir.dt.float16
RELU_FN = mybir.ActivationFunctionType.Relu

R = 512          # n_rec
NIN = 128        # n_in
RC = 4           # r chunks
B = 64           # batch (full, per window)
N_CORES = 8
NV = 2           # windows per core
NWIN = N_CORES * NV
T_FULL = 1000
BURN = int(os.environ.get("BIO_BURN", "48"))  # burn-in steps
WSTARTS = [(w * T_FULL) // NWIN for w in range(NWIN + 1)]
WLEN = max(WSTARTS[w + 1] - WSTARTS[w] for w in range(NWIN))  # 63
T_LOC = WLEN + BURN       # local steps per window
ALPHA = 0.2
LEAK = 1.0 - ALPHA
Q = 40                    # accum-q rescale block (0.8^-39 ~ 6e3: fp16-safe)
QOFF = (0, Q // 2)        # per-window frame offset: restarts never coincide
U = 48                    # r' ring steps
CPS = NV * 256            # ring cols per step-slot
ZCH = 8                   # z-stream DMA chunk (steps)
ZLEAD = 40                # z chunks DMA'd this many steps ahead
OCH = 4                   # out-dump chunk (steps)


def build_nc(T=T_LOC):
    """Build the per-core Bass program."""
    nc = bacc.Bacc()

    # host-precomputed pre-scaled z streams, fp16, transposed layout
    # [p, v, t, c]: c = m*64+b; c 0:128 = delta-enc (bank A), 128:256 = raw
    z_d = nc.dram_tensor("z16", [128, NV, T, 256], F16,
                         kind="ExternalInput").ap()
    w_d = nc.dram_tensor("w16", [R, R], F16, kind="ExternalInput").ap()
    # raw r' dump: [p, t, v*256 + m*64 + b] fp16; host integrates h
    o_d = nc.dram_tensor("outT16", [128, T, CPS], F16,
                         kind="ExternalOutput").ap()

    with tile.TileContext(nc) as tc, ExitStack() as ctx:
        const = ctx.enter_context(tc.tile_pool(name="const", bufs=1))
        big = ctx.enter_context(tc.tile_pool(name="big", bufs=1))
        sp = ctx.enter_context(tc.tile_pool(name="sp", bufs=2))

        ident16 = const.tile([128, 128], F16)
        make_identity(nc, ident16[:, :])
        zero16 = const.tile([128, 128], F16)
        nc.vector.memset(zero16[:, :], 0.0)

        w16 = big.tile([128, RC * R], F16)
        zt = big.tile([128, NV * T * 256], F16)   # z streams, full resident
        rr = big.tile([128, U * CPS], F16)        # r' ring

        pools = [ctx.enter_context(
            tc.tile_pool(name=f"ps{n}", bufs=1, space="PSUM"))
            for n in ("a0", "c0", "a1", "c1")]
        pst = [p.tile([128, 512], F32, name=f"ps{i}", tag=f"ps{i}")
               for i, p in enumerate(pools)]
        psA = [pst[0], pst[2]]
        psC = [pst[1], pst[3]]

        # ---- DMA helpers: dedicated queues per stream ----
        qin = [nc.gpsimd, nc.sync]

        def emit_in(v, t0, t1):
            t1 = min(t1, T)
            if t0 >= t1:
                return
            qin[v].dma_start(
                out=zt[:, (v * T + t0) * 256:(v * T + t1) * 256]
                .rearrange("p (t c) -> p t c", c=256),
                in_=z_d[:, v, t0:t1, :])

        oq = [0]

        def emit_out(a, e):
            s0 = (a % U) * CPS
            q = qin[oq[0] % 2]
            oq[0] += 1
            q.dma_start(
                out=o_d[:, a:e, :],
                in_=rr[:, s0:s0 + (e - a) * CPS]
                .rearrange("p (t c) -> p t c", c=CPS))

        # startup: first chunks + single-trigger weight load, few big DMAs
        emit_in(0, 0, 8)
        nc.sync.dma_start(
            out=w16[:, :].rearrange("p (k c) -> p k c", c=R),
            in_=w_d.rearrange("(k p) c -> p k c", p=128))
        emit_in(1, 0, 8)
        emit_in(0, 8, 24)
        emit_in(1, 8, 24)
        for t0 in range(24, min(ZLEAD, T), ZCH):
            for v in range(NV):
                emit_in(v, t0, t0 + ZCH)

        mm = nc.tensor.matmul

        def block(v, t):
            pa, pc = psA[v], psC[v]
            base = (v * T + t) * 256
            zA = zt[:, base:base + 128]
            zC = zt[:, base + 128:base + 256]
            so = (t % U) * CPS + v * 256
            outA = rr[:, so:so + 128]
            outC = rr[:, so + 128:so + 256]
            pr = ((t - 1) % U) * CPS + v * 256

            def rk(k):
                return rr[:, pr + k * 64:pr + (k + 1) * 64]

            if t == 0:
                mm(pa[:, :128], lhsT=ident16[:, :], rhs=zA,
                   start=True, stop=True, skip_group_check=True)
                mm(pc[:, :128], lhsT=ident16[:, :], rhs=zero16[:, :],
                   start=True, stop=True, skip_group_check=True)
            else:
                if (t + QOFF[v]) % Q == 0:
                    # re-inject q at true scale (q := 0.8^Q * q)
                    s16a = sp.tile([128, 128], F16, tag=f"s16a{v}")
                    s16c = sp.tile([128, 128], F16, tag=f"s16c{v}")
                    nc.scalar.mul(out=s16a[:, :], in_=pa[:, :128],
                                  mul=float(LEAK ** Q))
                    nc.vector.tensor_scalar_mul(s16c[:, :], pc[:, :128],
                                                float(LEAK ** Q))
                    mm(pa[:, :128], lhsT=ident16[:, :], rhs=s16a[:, :],
                       start=True, stop=False, skip_group_check=True)
                    mm(pc[:, :128], lhsT=ident16[:, :], rhs=s16c[:, :],
                       start=True, stop=False, skip_group_check=True)

                def kmm(m, k, stop=False):
                    ps = pa if m < 2 else pc
                    off = (m % 2) * 64
                    mm(ps[:, off:off + 64],
                       lhsT=w16[:, k * R + m * 128:k * R + (m + 1) * 128],
                       rhs=rk(k), start=False, stop=stop,
                       skip_group_check=True)

                mm(pa[:, :128], lhsT=ident16[:, :], rhs=zA,
                   start=False, stop=False, skip_group_check=True)
                kmm(0, 0); kmm(1, 0); kmm(0, 1); kmm(1, 1)
                kmm(2, 0); kmm(3, 0); kmm(2, 1); kmm(3, 1)
                kmm(0, 2); kmm(1, 2); kmm(0, 3); kmm(1, 3, stop=True)
                kmm(2, 2); kmm(3, 2); kmm(2, 3); kmm(3, 3, stop=True)

            jp = (t + QOFF[v]) % Q
            jn = (t + 1 + QOFF[v]) % Q
            c0 = float(ALPHA * LEAK ** (jp - jn))
            nc.scalar.activation(out=outA, in_=pa[:, :128], func=RELU_FN,
                                 scale=c0)
            nc.vector._custom_dve(RELU2, out=outC, in0=pc[:, :128],
                                  in1=zC, s0=c0, s1=c0)

        for t in range(T):
            tp = t + ZLEAD
            if tp % ZCH == 0 and tp < T:
                emit_in(0, tp, tp + ZCH)
                emit_in(1, tp, tp + ZCH)
            block(0, t)
            block(1, t)
            if (t + 1) % OCH == 0 or t == T - 1:
                a = (t // OCH) * OCH
                emit_out(a, t + 1)

    nc.compile()
    return nc


def host_prep(x, w_in, w_rec, b_rec, ei_mask, autapse_mask, noise):
    """Host-side weight prep + window shard + pre-scaled fp16 z streams.

    z = x@w_in + noise + b_rec. Bank-A half (r 0:256): delta-encoded
    (z_t - 0.8 z_{t-1}) for psum injection. Bank-C half (r 256:512):
    raw (added at the DVE relu). Both scaled by 0.8^-(t % Q).
    """
    ei = np.diagonal(np.asarray(ei_mask)).astype(np.float32)
    w_eff = ei[:, None] * (np.asarray(w_rec) * np.asarray(autapse_mask))
    w16 = w_eff.astype(np.float16)
    x = np.asarray(x, dtype=np.float32)
    z = (x.reshape(-1, NIN) @ np.asarray(w_in, dtype=np.float32)).reshape(
        B, T_FULL, R)
    z += np.asarray(noise, dtype=np.float32)
    z += np.asarray(b_rec, np.float32)
    in_maps = []
    for c in range(N_CORES):
        zwins = []
        for v in range(NV):
            jscale = (LEAK ** -((np.arange(T_LOC) + QOFF[v]) % Q)
                      ).astype(np.float32)
            w = NV * c + v
            t0 = WSTARTS[w] - BURN
            zp = np.zeros((B, T_LOC, R), np.float32)
            s = max(t0, 0)
            zp[:, s - t0:] = z[:, s:t0 + T_LOC]
            # [p, t, m, b]
            zt4 = zp.reshape(B, T_LOC, RC, 128).transpose(3, 1, 2, 0)
            dA = zt4[:, :, 0:2, :].copy()
            dA[:, 1:] -= LEAK * dA[:, :-1].copy()
            dA *= jscale[None, :, None, None]
            zC = zt4[:, :, 2:4, :] * jscale[None, :, None, None]
            zwins.append(np.concatenate(
                [dA.reshape(128, T_LOC, 128), zC.reshape(128, T_LOC, 128)],
                axis=2))
        z16 = np.ascontiguousarray(
            np.stack(zwins, axis=1).astype(np.float16))
        in_maps.append({"z16": z16, "w16": w16})
    return in_maps, w_eff.astype(np.float32)


def _integrate(dump):
    """dump: [128, T, CPS] fp16 per core -> list of NV h arrays
    (B, T_LOC, R) f32 via host leaky integration."""
    hs = []
    for v in range(NV):
        jn = ((np.arange(T_LOC) + 1 + QOFF[v]) % Q).astype(np.float32)
        sc = (LEAK ** jn).astype(np.float32)
        rp = dump[:, :, v * 256:(v + 1) * 256].astype(np.float32)
        # [p, t, m, b] -> [b, t, r]
        ar = rp.reshape(128, T_LOC, RC, B).transpose(3, 1, 2, 0).reshape(
            B, T_LOC, R)
        ar *= sc[None, :, None]
        h = np.empty_like(ar)
        acc = np.zeros((B, R), np.float32)
        for t in range(T_LOC):
            acc = LEAK * acc + ar[:, t]
            h[:, t] = acc
        hs.append(h)
    return hs


def reference_np(x, w_in, b_rec, w_eff, noise, T=None):
    """Numpy reference for dev checks (f32)."""
    x = np.asarray(x, np.float32)
    if T is None:
        T = x.shape[1]
    z = np.einsum("bti,ir->btr", x[:, :T], np.asarray(w_in)) \
        + np.asarray(noise)[:, :T] + np.asarray(b_rec)
    h = np.zeros((x.shape[0], w_eff.shape[0]), np.float32)
    outs = []
    for t in range(T):
        pre = z[:, t] + h @ w_eff
        h = LEAK * h + ALPHA * np.maximum(pre, 0.0)
        outs.append(h.copy())
    return np.stack(outs, axis=1)


# ---------------------------------------------------------------------------
# harness entry point
# ---------------------------------------------------------------------------
_NC_CACHE = {}


def kernel(x, w_in, w_rec, b_rec, ei_mask, autapse_mask, noise):
    from concourse.bass_utils import run_bass_kernel_spmd

    x = np.asarray(x)
    T = x.shape[1]
    in_maps, _ = host_prep(x, w_in, w_rec, b_rec, ei_mask, autapse_mask, noise)
    if T not in _NC_CACHE:
        _NC_CACHE[T] = build_nc()
    nc = _NC_CACHE[T]
    res = run_bass_kernel_spmd(nc, in_maps, core_ids=list(range(N_CORES)))
    out = np.empty((x.shape[0], T, R), np.float32)
    for c in range(N_CORES):
        hs = _integrate(res.results[c]["outT16"])
        for v in range(NV):
            w = NV * c + v
            a, e = WSTARTS[w], WSTARTS[w + 1]
            out[:, a:e] = hs[v][:, BURN:BURN + (e - a)]
    return out


# revision 14
# speedup vs baseline: 1.3508x; 1.0322x over previous
"""BioRNN Trainium2 kernel (dev module).

Sharding: 16 time-windows (2 per core, ~63 output steps each), full
batch 64 per window. The leak (0.8/step) forgets initial state: a
32-step burn-in from h=0 reproduces the true state (window 0 pads with
zeros, exact). Per core: 2 windows x T_LOC=95 steps, emitted half-step
out of phase so one window's relu handoff hides under the other's
matmuls.

accum-q recurrence, v-part only in psum: q = 0.8^-j * v_t within a
Q=32 block (j = t % Q), v_t = h_{t-1} @ w_eff. Per window-step:
    1 inj matmul   identity @ dz_t (bank A only; N=128)
    16 W matmuls   r'_t @ w_eff (N=64)
where dz is the HOST-precomputed delta of z = x@w_in + noise + b_rec
for the bank-A half (r 0:256), pre-scaled by 0.8^-j, fp16, transposed.
Bank C's z half never enters psum: it rides in through a 2-input DVE
relu. The two relus run on DIFFERENT engines to halve the handoff
serialization:
    bank A (r 0:256):  ACT   r'a = relu(qA * c0)         (z in psum)
    bank C (r 256:512): DVE  r'c = relu(qC*c0 + z*c0)    (z from SBUF)
with c0 = 0.2*0.8^(jp-jn). Every Q steps each bank is re-injected at
true scale (ACT/DVE mul -> fp16 -> identity matmul, start=True).

h is NOT computed on device: h_t = 0.8 h_{t-1} + 0.8^jn r'_t is a
leaky integration the HOST does in f32 over the dumped r' stream
(more accurate than the old fp16 device ring). r' ring chunks DMA to
DRAM in native (p, t, v, m, b) fp16 layout; host un-transposes,
rescales by 0.8^jn(t), and lfilters to h.
"""

import os
import numpy as np
from contextlib import ExitStack

import concourse.bass as bass
import concourse.mybir as mybir
import concourse.tile as tile
from concourse import bacc
from concourse import dve_ops
from concourse.dve_spec import (
    Spec, Src0, Src1, C0, C1, relu as _dve_relu_expr, lower,
)
from concourse.dve_uop import DveOpSpec
from concourse.masks import make_identity


def _register_dve(name, body, ref, rd1=True):
    """Register a custom DVE op (idempotent)."""
    for o in dve_ops.OPS:
        if o.name == name:
            return o
    opcode = max(dve_ops._SUB_OPCODE_FOR_NAME.values()) + 1
    assert opcode < 0x20
    dve_ops._SUB_OPCODE_FOR_NAME[name] = opcode
    spec = Spec(body=body, reference=ref)
    shas = {}
    for ver in ("v3", "v4"):
        s = DveOpSpec(name=name, opcode=opcode, uops=lower(spec, ver=ver),
                      rd1_en=rd1)
        shas[ver] = s.sha(ver)
    op = dve_ops.DveOp(name, spec, subdim=False, uops_sha=shas)
    dve_ops.OPS.append(op)
    dve_ops.CUSTOM_DVE_SPECS[name] = spec
    return op


def _f32(a):
    return a.astype(np.float32).reshape(a.shape[0], -1)


def _ref_relu2(in0, in1, c0, c1, c2):
    s = np.maximum(np.nan_to_num(_f32(in0) * c0 + _f32(in1) * c1,
                                 nan=0.0, posinf=np.inf, neginf=-np.inf), 0)
    return s.reshape(in0.shape)


RELU2 = _register_dve("RELU2_BIO", _dve_relu_expr(Src0 * C0 + Src1 * C1),
                      _ref_relu2, rd1=True)

F32 = mybir.dt.float32
F16 = mybir.dt.float16
RELU_FN = mybir.ActivationFunctionType.Relu

R = 512          # n_rec
NIN = 128        # n_in
RC = 4           # r chunks
B = 64           # batch (full, per window)
N_CORES = 8
NV = 2           # windows per core
NWIN = N_CORES * NV
T_FULL = 1000
BURN = int(os.environ.get("BIO_BURN", "48"))  # burn-in steps
WSTARTS = [(w * T_FULL) // NWIN for w in range(NWIN + 1)]
WLEN = max(WSTARTS[w + 1] - WSTARTS[w] for w in range(NWIN))  # 63
T_LOC = WLEN + BURN       # local steps per window
ALPHA = 0.2
LEAK = 1.0 - ALPHA
Q = 40                    # accum-q rescale block (0.8^-39 ~ 6e3: fp16-safe)
QOFF = (0, Q // 2)        # per-window frame offset: restarts never coincide
U = 48                    # r' ring steps
CPS = NV * 256            # ring cols per step-slot
ZCH = 8                   # z-stream DMA chunk (steps)
ZLEAD = 40                # z chunks DMA'd this many steps ahead
OCH = 4                   # out-dump chunk (steps)


def build_nc(T=T_LOC):
    """Build the per-core Bass program."""
    nc = bacc.Bacc()

    # host-precomputed pre-scaled z streams, fp16, transposed layout
    # [p, v, t, c]: c = m*64+b; c 0:128 = delta-enc (bank A), 128:256 = raw
    z_d = nc.dram_tensor("z16", [128, NV, T, 256], F16,
                         kind="ExternalInput").ap()
    w_d = nc.dram_tensor("w16", [R, R], F16, kind="ExternalInput").ap()
    # raw r' dump: [p, t, v*256 + m*64 + b] fp16; host integrates h
    o_d = nc.dram_tensor("outT16", [128, T, CPS], F16,
                         kind="ExternalOutput").ap()

    with tile.TileContext(nc) as tc, ExitStack() as ctx:
        const = ctx.enter_context(tc.tile_pool(name="const", bufs=1))
        big = ctx.enter_context(tc.tile_pool(name="big", bufs=1))
        sp = ctx.enter_context(tc.tile_pool(name="sp", bufs=2))

        ident16 = const.tile([128, 128], F16)
        make_identity(nc, ident16[:, :])
        zero16 = const.tile([128, 128], F16)
        nc.vector.memset(zero16[:, :], 0.0)

        w16 = big.tile([128, RC * R], F16)
        zt = big.tile([128, NV * T * 256], F16)   # z streams, full resident
        rr = big.tile([128, U * CPS], F16)        # r' ring

        pools = [ctx.enter_context(
            tc.tile_pool(name=f"ps{n}", bufs=1, space="PSUM"))
            for n in ("a0", "c0", "a1", "c1")]
        pst = [p.tile([128, 512], F32, name=f"ps{i}", tag=f"ps{i}")
               for i, p in enumerate(pools)]
        psA = [pst[0], pst[2]]
        psC = [pst[1], pst[3]]

        # ---- DMA helpers: in-triggers must never sit behind sem-gated
        # out-triggers (head-of-line), so the streams get separate queues
        def emit_in(v, t0, t1):
            t1 = min(t1, T)
            if t0 >= t1:
                return
            nc.gpsimd.dma_start(
                out=zt[:, (v * T + t0) * 256:(v * T + t1) * 256]
                .rearrange("p (t c) -> p t c", c=256),
                in_=z_d[:, v, t0:t1, :])

        def emit_out(a, e):
            s0 = (a % U) * CPS
            nc.sync.dma_start(
                out=o_d[:, a:e, :],
                in_=rr[:, s0:s0 + (e - a) * CPS]
                .rearrange("p (t c) -> p t c", c=CPS))

        # startup: first chunks + single-trigger weight load, few big DMAs
        emit_in(0, 0, 8)
        nc.sync.dma_start(
            out=w16[:, :].rearrange("p (k c) -> p k c", c=R),
            in_=w_d.rearrange("(k p) c -> p k c", p=128))
        emit_in(1, 0, 8)
        emit_in(0, 8, 24)
        emit_in(1, 8, 24)
        for t0 in range(24, min(ZLEAD, T), ZCH):
            for v in range(NV):
                emit_in(v, t0, t0 + ZCH)

        mm = nc.tensor.matmul

        def block(v, t):
            pa, pc = psA[v], psC[v]
            base = (v * T + t) * 256
            zA = zt[:, base:base + 128]
            zC = zt[:, base + 128:base + 256]
            so = (t % U) * CPS + v * 256
            outA = rr[:, so:so + 128]
            outC = rr[:, so + 128:so + 256]
            pr = ((t - 1) % U) * CPS + v * 256

            def rk(k):
                return rr[:, pr + k * 64:pr + (k + 1) * 64]

            if t == 0:
                mm(pa[:, :128], lhsT=ident16[:, :], rhs=zA,
                   start=True, stop=True, skip_group_check=True)
                mm(pc[:, :128], lhsT=ident16[:, :], rhs=zero16[:, :],
                   start=True, stop=True, skip_group_check=True)
            else:
                if (t + QOFF[v]) % Q == 0:
                    # re-inject q at true scale (q := 0.8^Q * q)
                    s16a = sp.tile([128, 128], F16, tag=f"s16a{v}")
                    s16c = sp.tile([128, 128], F16, tag=f"s16c{v}")
                    nc.scalar.mul(out=s16a[:, :], in_=pa[:, :128],
                                  mul=float(LEAK ** Q))
                    nc.vector.tensor_scalar_mul(s16c[:, :], pc[:, :128],
                                                float(LEAK ** Q))
                    mm(pa[:, :128], lhsT=ident16[:, :], rhs=s16a[:, :],
                       start=True, stop=False, skip_group_check=True)
                    mm(pc[:, :128], lhsT=ident16[:, :], rhs=s16c[:, :],
                       start=True, stop=False, skip_group_check=True)

                def kmm(m, k, stop=False):
                    ps = pa if m < 2 else pc
                    off = (m % 2) * 64
                    mm(ps[:, off:off + 64],
                       lhsT=w16[:, k * R + m * 128:k * R + (m + 1) * 128],
                       rhs=rk(k), start=False, stop=stop,
                       skip_group_check=True)

                mm(pa[:, :128], lhsT=ident16[:, :], rhs=zA,
                   start=False, stop=False, skip_group_check=True)
                kmm(0, 0); kmm(1, 0); kmm(0, 1); kmm(1, 1)
                kmm(2, 0); kmm(3, 0); kmm(2, 1); kmm(3, 1)
                kmm(0, 2); kmm(1, 2); kmm(0, 3); kmm(1, 3, stop=True)
                kmm(2, 2); kmm(3, 2); kmm(2, 3); kmm(3, 3, stop=True)

            jp = (t + QOFF[v]) % Q
            jn = (t + 1 + QOFF[v]) % Q
            c0 = float(ALPHA * LEAK ** (jp - jn))
            nc.scalar.activation(out=outA, in_=pa[:, :128], func=RELU_FN,
                                 scale=c0)
            nc.vector._custom_dve(RELU2, out=outC, in0=pc[:, :128],
                                  in1=zC, s0=c0, s1=c0)

        for t in range(T):
            tp = t + ZLEAD
            if tp % ZCH == 0 and tp < T:
                emit_in(0, tp, tp + ZCH)
                emit_in(1, tp, tp + ZCH)
            block(0, t)
            block(1, t)
            if (t + 1) % OCH == 0 or t == T - 1:
                a = (t // OCH) * OCH
                emit_out(a, t + 1)

    nc.compile()
    return nc


def host_prep(x, w_in, w_rec, b_rec, ei_mask, autapse_mask, noise):
    """Host-side weight prep + window shard + pre-scaled fp16 z streams.

    z = x@w_in + noise + b_rec. Bank-A half (r 0:256): delta-encoded
    (z_t - 0.8 z_{t-1}) for psum injection. Bank-C half (r 256:512):
    raw (added at the DVE relu). Both scaled by 0.8^-(t % Q).
    """
    ei = np.diagonal(np.asarray(ei_mask)).astype(np.float32)
    w_eff = ei[:, None] * (np.asarray(w_rec) * np.asarray(autapse_mask))
    w16 = w_eff.astype(np.float16)
    x = np.asarray(x, dtype=np.float32)
    z = (x.reshape(-1, NIN) @ np.asarray(w_in, dtype=np.float32)).reshape(
        B, T_FULL, R)
    z += np.asarray(noise, dtype=np.float32)
    z += np.asarray(b_rec, np.float32)
    in_maps = []
    for c in range(N_CORES):
        zwins = []
        for v in range(NV):
            jscale = (LEAK ** -((np.arange(T_LOC) + QOFF[v]) % Q)
                      ).astype(np.float32)
            w = NV * c + v
            t0 = WSTARTS[w] - BURN
            zp = np.zeros((B, T_LOC, R), np.float32)
            s = max(t0, 0)
            zp[:, s - t0:] = z[:, s:t0 + T_LOC]
            # [p, t, m, b]
            zt4 = zp.reshape(B, T_LOC, RC, 128).transpose(3, 1, 2, 0)
            dA = zt4[:, :, 0:2, :].copy()
            dA[:, 1:] -= LEAK * dA[:, :-1].copy()
            dA *= jscale[None, :, None, None]
            zC = zt4[:, :, 2:4, :] * jscale[None, :, None, None]
            zwins.append(np.concatenate(
                [dA.reshape(128, T_LOC, 128), zC.reshape(128, T_LOC, 128)],
                axis=2))
        z16 = np.ascontiguousarray(
            np.stack(zwins, axis=1).astype(np.float16))
        in_maps.append({"z16": z16, "w16": w16})
    return in_maps, w_eff.astype(np.float32)


def _integrate(dump):
    """dump: [128, T, CPS] fp16 per core -> list of NV h arrays
    (B, T_LOC, R) f32 via host leaky integration."""
    hs = []
    for v in range(NV):
        jn = ((np.arange(T_LOC) + 1 + QOFF[v]) % Q).astype(np.float32)
        sc = (LEAK ** jn).astype(np.float32)
        rp = dump[:, :, v * 256:(v + 1) * 256].astype(np.float32)
        # [p, t, m, b] -> [b, t, r]
        ar = rp.reshape(128, T_LOC, RC, B).transpose(3, 1, 2, 0).reshape(
            B, T_LOC, R)
        ar *= sc[None, :, None]
        h = np.empty_like(ar)
        acc = np.zeros((B, R), np.float32)
        for t in range(T_LOC):
            acc = LEAK * acc + ar[:, t]
            h[:, t] = acc
        hs.append(h)
    return hs


def reference_np(x, w_in, b_rec, w_eff, noise, T=None):
    """Numpy reference for dev checks (f32)."""
    x = np.asarray(x, np.float32)
    if T is None:
        T = x.shape[1]
    z = np.einsum("bti,ir->btr", x[:, :T], np.asarray(w_in)) \
        + np.asarray(noise)[:, :T] + np.asarray(b_rec)
    h = np.zeros((x.shape[0], w_eff.shape[0]), np.float32)
    outs = []
    for t in range(T):
        pre = z[:, t] + h @ w_eff
        h = LEAK * h + ALPHA * np.maximum(pre, 0.0)
        outs.append(h.copy())
    return np.stack(outs, axis=1)


# ---------------------------------------------------------------------------
# harness entry point
# ---------------------------------------------------------------------------
_NC_CACHE = {}


def kernel(x, w_in, w_rec, b_rec, ei_mask, autapse_mask, noise):
    from concourse.bass_utils import run_bass_kernel_spmd

    x = np.asarray(x)
    T = x.shape[1]
    in_maps, _ = host_prep(x, w_in, w_rec, b_rec, ei_mask, autapse_mask, noise)
    if T not in _NC_CACHE:
        _NC_CACHE[T] = build_nc()
    nc = _NC_CACHE[T]
    res = run_bass_kernel_spmd(nc, in_maps, core_ids=list(range(N_CORES)))
    out = np.empty((x.shape[0], T, R), np.float32)
    for c in range(N_CORES):
        hs = _integrate(res.results[c]["outT16"])
        for v in range(NV):
            w = NV * c + v
            a, e = WSTARTS[w], WSTARTS[w + 1]
            out[:, a:e] = hs[v][:, BURN:BURN + (e - a)]
    return out
